# revision 48
# baseline (speedup 1.0000x reference)
"""Complex-valued dot-product attention (B=4, S=4096, D=64) on 8 TRN2 cores.

Harness entry: kernel(**inputs) -> np.ndarray [2, 4, 4096, 64] fp32.

Math (per batch): s = (q_re + i q_im)(k_re + i k_im)^T / 8,
w = softmax(|s|, axis=keys), out = stack(w @ v_re, w @ v_im).

Sharding: core c = (batch b = c//2, key half h = c%2). Each core computes a
partial softmax (flash-style) over its 2048 keys for all 4096 queries:
  OT = sum_k E^T [v_re|v_im],  R = sum_k E^T,  E = exp(|s|/8 - 6)
and the gather step combines o = (OT_0 + OT_1) / (R_0 + R_1) exactly (the
-6 bias is a shared constant so no per-shard max bookkeeping is needed).

Device kernel (per core): scores are built transposed, s^T [k_part, q_free],
via fp16 matmuls contracting 128 re/im-concatenated channels. Per k-tile a
custom fused DVE op (out = in0^2 + in1^2) drains both PSUM banks into
|s_raw|^2 in fp16. The whole  sqrt -> exp  chain then runs as a SINGLE
scalar-engine pass: the kernel compiles against a patched PWP activation
table in which the `exp` buckets are rewritten (same expansion points,
Taylor coefficients of g(t) = exp(sqrt(t) - 6)), so one ACT instruction
computes E = exp(|s_raw|/8 - 6) directly from |s_raw|^2 with scale=1/64.
The softmax row-sum (ones-matmul) and PV matmuls stream E^T back through
the tensor engine, PSUM-accumulated across the 16 k-tiles.
"""

import hashlib
import json
import os
import shutil

import numpy as np

import concourse.bacc as bacc
import concourse.bass as bass
import concourse.mybir as mybir
import concourse.tile as tile

F16 = mybir.dt.float16
F32 = mybir.dt.float32
AF = mybir.ActivationFunctionType

D = 64
SCALE = 1.0 / np.sqrt(np.float32(D))
EXP_BIAS = -6.0
M2_FLOOR = 2.0 ** -10  # keeps table input off the small-signal path

_OPS = {}
_CACHE = {}


# ------------------------------------------------------- custom DVE op
def _register(name, make_spec):
    from concourse import dve_ops
    from concourse.dve_ops import DveOp
    from concourse.dve_spec import lower, _has_src1
    from concourse.dve_uop import DveOpSpec

    if name in _OPS:
        return _OPS[name]
    for op in dve_ops.OPS:
        if op.name == name:
            _OPS[name] = op
            return op
    spec = make_spec()
    row = dve_ops._CUSTOM_DVE_ROW_BASE + len(dve_ops.OPS)
    assert row < 0x20
    dve_ops._SUB_OPCODE_FOR_NAME[name] = row
    shas = {}
    for ver in ("v3", "v4"):
        ds = DveOpSpec(
            name=name, opcode=row, uops=lower(spec, ver=ver), rd1_en=_has_src1(spec)
        )
        shas[ver] = ds.sha(ver)
    op = DveOp(name, spec, subdim=False, uops_sha=shas)
    dve_ops.OPS.append(op)
    dve_ops.CUSTOM_DVE_SPECS[name] = spec
    _OPS[name] = op
    return op


def register_sumsq():
    from concourse.dve_spec import Spec, Src0, Src1, sq

    return _register(
        "SUMSQ_ANT",
        lambda: Spec(
            body=sq(Src0) + Src1,
            reference=lambda in0, in1, s0, s1, imm2: (
                in0.astype(np.float32) ** 2 + in1.astype(np.float32)
            ),
        ),
    )


def register_sqdrain():
    from concourse.dve_spec import Spec, Src0, sq

    return _register(
        "SQDRAIN_ANT",
        lambda: Spec(
            body=sq(Src0),
            reference=lambda in0, in1, s0, s1, imm2: (
                in0.astype(np.float32) ** 2
            ),
        ),
    )


# ------------------------------------------------- patched activation table
# Rewrites every `exp` bucket (identified by its Taylor signature
# d0=e^a, d1=e^a, d2=e^a/2) so the scalar engine's Exp computes
# g(t) = exp(sqrt(t) - 6) for t > 0 and e^-6 for t <= 0.
def _patch_exp_buckets(raw):
    arr = np.frombuffer(raw, dtype=np.float32).reshape(-1, 8).copy()
    a = arr[:, 4].astype(np.float64)
    d0 = arr[:, 0].astype(np.float64)
    d1 = arr[:, 1].astype(np.float64)
    d2 = arr[:, 2].astype(np.float64)
    with np.errstate(over="ignore", invalid="ignore"):
        ea = np.exp(a)
        is_exp = (
            np.isfinite(ea)
            & (d0 > 0)
            & (np.abs(d0 - ea) <= 1e-5 * ea)
            & (np.abs(d1 - ea) <= 1e-5 * ea)
            & (np.abs(2.0 * d2 - ea) <= 1e-4 * ea)
        )
    pos = is_exp & (a > 0)
    A = a[pos]
    sA = np.sqrt(A)
    E = np.exp(sA + EXP_BIAS)
    arr[pos, 0] = E.astype(np.float32)
    arr[pos, 1] = (E / (2 * sA)).astype(np.float32)
    arr[pos, 2] = ((E * (1 / (4 * A) - 1 / (4 * A**1.5))) / 2.0).astype(np.float32)
    arr[pos, 3] = (
        (E * (1 / (8 * A**1.5) - 3 / (8 * A**2) + 3 / (8 * A**2.5))) / 6.0
    ).astype(np.float32)
    neg = is_exp & (a <= 0)
    arr[neg, 0] = np.float32(np.exp(EXP_BIAS))
    arr[neg, 1] = 0.0
    arr[neg, 2] = 0.0
    arr[neg, 3] = 0.0
    return arr.tobytes(), int(pos.sum())


def _build_table_dir():
    """Copy the compiler's default PWP table dir, patching exp buckets.

    Returns (table_dir, short content tag). Idempotent per content tag.
    """
    from neuronxcc.driver.Job import Job
    from neuronxcc.driver.jobs.support.FindActInfo import findActInfoFile

    src_json = os.environ.get("BASS_ACT_ROOT_JSON_PATH") or findActInfoFile(
        Job.getPackageDir(), "core_v4"
    )
    src = os.path.dirname(src_json)
    with open(src_json) as f:
        info = json.load(f)

    h = hashlib.sha256(b"expsqrt_v2")
    patches = {}
    n_sets = 0
    for s in info["act_func_sets"]:
        if "exp" not in s.get("act", {}):
            continue
        p = os.path.join(src, s["bkt_bin"])
        with open(p, "rb") as f:
            raw = f.read()
        patched, n_pos = _patch_exp_buckets(raw)
        assert n_pos > 100, f"{s['name']}: only {n_pos} exp buckets matched"
        patches[s["bkt_bin"]] = patched
        h.update(patched)
        n_sets += 1
    assert n_sets >= 1, "no exp-containing activation table sets found"
    tag = h.hexdigest()[:10]

    dst = f"/tmp/acttab_{tag}"
    if not os.path.exists(os.path.join(dst, os.path.basename(src_json))):
        tmp = dst + f".tmp{os.getpid()}"
        if os.path.exists(tmp):
            shutil.rmtree(tmp)
        shutil.copytree(src, tmp)
        os.chmod(tmp, 0o755)
        for fn in os.listdir(tmp):
            os.chmod(os.path.join(tmp, fn), 0o644)
        for fn, data in patches.items():
            with open(os.path.join(tmp, fn), "wb") as f:
                f.write(data)
        if os.path.exists(dst):
            shutil.rmtree(tmp)
        else:
            os.rename(tmp, dst)
    return os.path.join(dst, os.path.basename(src_json)), tag


# ---------------------------------------------------------------- device kernel
def build_kernel(SQ=4096, SK=2048, CH=512, dve_drain_mod=3):
    """dve_drain_mod: im^2 PSUM drains go to DVE (instead of ACT Square)
    for k-tiles with kt % dve_drain_mod == dve_drain_mod - 1. Balances the
    scalar and vector engines; dual-PSUM DVE reads are illegal so the
    re^2+im^2 combine always reads re from PSUM and im^2 from SBUF."""
    table_json, tag = _build_table_dir()
    os.environ["BASS_ACT_ROOT_JSON_PATH"] = table_json

    sumsq = register_sumsq()
    sqdrain = register_sqdrain()
    KT = SK // 128
    NITER = SQ // (2 * CH)
    W = 2 * CH
    REG = KT * W

    def mm_reuse(out, lhsT, rhs, start, stop):
        """Matmul that reuses the stationary operand loaded by the previous
        (self-loading) matmul — walrus skips the LDWEIGHTS re-emission."""
        eng = nc.tensor
        ifmap_ap = eng.lower_ap(rhs.opt(frozenset({0})), opt=False)
        weights_ap = eng.lower_ap(lhsT.opt(frozenset({0})), opt=False,
                                  for_matmul_weights=True)
        out_ap = eng.lower_ap(out)
        return eng.add_instruction(mybir.InstMatmult(
            name=nc.get_next_instruction_name(),
            replication_resolution=0, replication_shift_amnt=0,
            replication_num_rows=0,
            start_tensor_calc=start, stop_tensor_calc=stop,
            ins=[ifmap_ap, weights_ap], outs=[out_ap],
            ldweights=False,
            tile_position=(lhsT.base_partition(), out.base_partition()),
            tile_size=(128, round_up_pe(out.partition_size())),
        ))

    def round_up_pe(size):
        for v in (32, 64, 128):
            if v >= size:
                return v
        raise AssertionError(size)

    nc = bacc.Bacc("TRN2", target_bir_lowering=False)
    # Tensor names carry the table tag: the PJRT compile cache is keyed on
    # the BIR, and the activation table (env var) is not part of that key.
    qc_d = nc.dram_tensor(f"qc_{tag}", [128, SQ], F16, kind="ExternalInput")
    kc1_d = nc.dram_tensor(f"kc1_{tag}", [128, SK], F16, kind="ExternalInput")
    kc2_d = nc.dram_tensor(f"kc2_{tag}", [128, SK], F16, kind="ExternalInput")
    vc_d = nc.dram_tensor(f"vc_{tag}", [KT, 128, 128], F16, kind="ExternalInput")
    ot_d = nc.dram_tensor(f"ot_{tag}", [128, SQ], F32, kind="ExternalOutput")
    r_d = nc.dram_tensor(f"r_{tag}", [1, SQ], F32, kind="ExternalOutput")

    with tile.TileContext(nc) as tc:
        with (
            tc.tile_pool(name="singles", bufs=1) as singles,
            tc.tile_pool(name="stage", bufs=2) as stage,
            tc.tile_pool(name="outp", bufs=2) as outp,
            tc.tile_pool(name="ps_re", bufs=3, space="PSUM") as ps_re_pool,
            tc.tile_pool(name="ps_im", bufs=2, space="PSUM") as ps_im_pool,
            tc.tile_pool(name="ps_acc", bufs=1, space="PSUM") as ps_acc,
        ):
            # Inputs live in per-chunk tiles: tile-granular DMA dependencies
            # mean the first score matmul only waits for the first three
            # transfers, not a whole input tensor.
            GK = 4  # k-tiles per kc/vc chunk
            qc_t = [[singles.tile([128, CH], F16, name=f"qct{i}h{h}")
                     for h in range(2)] for i in range(NITER)]
            kc1_t = [singles.tile([128, GK * 128], F16, name=f"kc1t{i}")
                     for i in range(KT // GK)]
            kc2_t = [singles.tile([128, GK * 128], F16, name=f"kc2t{i}")
                     for i in range(KT // GK)]
            vc_t = [singles.tile([128, GK * 128], F16, name=f"vct{i}")
                    for i in range(KT // GK)]
            ones = singles.tile([128, 1], F16)
            ksl = lambda g: slice(g * GK * 128, (g + 1) * GK * 128)
            qsl = lambda i, h: slice(i * W + h * CH, i * W + (h + 1) * CH)
            # Input loads ordered by first use. Issuing a DMA costs the
            # issuing ENGINE ~600ns, so only the two earliest loads ride
            # the Scalar HW-DGE queue (Scalar is idle until the first
            # Square at ~9us); everything else stays on Sync.
            nc.sync.dma_start(kc2_t[0][:], kc2_d.ap()[:, ksl(0)])
            nc.scalar.dma_start(qc_t[0][0][:], qc_d.ap()[:, qsl(0, 0)])
            nc.scalar.dma_start(kc1_t[0][:], kc1_d.ap()[:, ksl(0)])
            nc.sync.dma_start(qc_t[0][1][:], qc_d.ap()[:, qsl(0, 1)])
            nc.scalar.dma_start(vc_t[0][:, 0:128], vc_d.ap()[0])
            for k in range(1, GK):
                nc.sync.dma_start(vc_t[0][:, k * 128 : (k + 1) * 128],
                                  vc_d.ap()[k])
            for g in range(1, KT // GK):
                nc.sync.dma_start(kc2_t[g][:], kc2_d.ap()[:, ksl(g)])
                nc.sync.dma_start(kc1_t[g][:], kc1_d.ap()[:, ksl(g)])
            for g in range(1, KT // GK):
                for k in range(GK):
                    nc.sync.dma_start(vc_t[g][:, k * 128 : (k + 1) * 128],
                                      vc_d.ap()[g * GK + k])
            for qi in range(1, NITER):
                for h in range(2):
                    nc.sync.dma_start(qc_t[qi][h][:], qc_d.ap()[:, qsl(qi, h)])
            nc.any.memset(ones[:], 1.0)
            g_bias = singles.tile([128, 1], F32)
            nc.any.memset(g_bias[:], M2_FLOOR)

            # PE warmup: the HAM clock gate defaults to 1.2 GHz and takes
            # ~3.4us of sustained activity to release. Burn tiny matmuls
            # into a scratch PSUM slot while the input DMAs stream so the
            # first real matmul runs at 2.4 GHz.
            warm_ps = ps_re_pool.tile([128, CH], F32, tag="re", name="warm_ps")
            for _ in range(40):
                nc.tensor.matmul(warm_ps[0:1, 0:1], ones[:, 0:1], ones[:, 0:1],
                                 start=True, stop=True)

            state = {}

            def emit_a_kt(qi, kt, v_buf):
                k_sl = slice((kt % GK) * 128, (kt % GK + 1) * 128)
                kc1k, kc2k = kc1_t[kt // GK], kc2_t[kt // GK]
                qch = [qc_t[qi][h][:] for h in range(2)]
                res, ims = [], []
                for half in range(2):
                    res.append(ps_re_pool.tile([128, CH], F32, tag="re",
                                               name=f"re_{qi}_{kt}_{half}"))
                    ims.append(ps_im_pool.tile([128, CH], F32, tag="im",
                                               name=f"im_{qi}_{kt}_{half}"))
                nc.tensor.matmul(ims[0][:], kc2k[:, k_sl], qch[0],
                                 start=True, stop=True)
                mm_reuse(ims[1][:], kc2k[:, k_sl], qch[1], True, True)
                nc.tensor.matmul(res[0][:], kc1k[:, k_sl], qch[0],
                                 start=True, stop=True)
                mm_reuse(res[1][:], kc1k[:, k_sl], qch[1], True, True)
                dve_drain = (kt % dve_drain_mod) == dve_drain_mod - 1
                for half in range(2):
                    reg = slice(kt * W + half * CH, kt * W + (half + 1) * CH)
                    if dve_drain:
                        nc.vector._custom_dve(
                            sqdrain, out=v_buf[:, reg], in0=ims[half][:]
                        )
                    else:
                        nc.scalar.activation(v_buf[:, reg], ims[half][:], AF.Square)
                    nc.vector._custom_dve(
                        sumsq, out=v_buf[:, reg], in0=res[half][:], in1=v_buf[:, reg]
                    )

            def emit_g_cols(qi, c0, ncols):
                # E = exp(sqrt(m2/64 + floor) - 6) via the patched exp table
                v_buf = state[qi]["v_buf"]
                sl = slice(c0, c0 + ncols)
                nc.scalar.activation(
                    v_buf[:, sl], v_buf[:, sl], AF.Exp,
                    scale=float(SCALE * SCALE), bias=g_bias[:],
                )

            def emit_d_kt(qi, kt, halves=(0, 1)):
                st = state[qi]
                if "ps_o" not in st:
                    st["ps_o"] = [
                        ps_acc.tile([128, CH], F32, tag="oA", name=f"ps_oA_{qi}"),
                        ps_acc.tile([128, CH], F32, tag="oB", name=f"ps_oB_{qi}"),
                    ]
                    # both row-sum accumulators share one PSUM bank: half 0
                    # lands on partition 0, half 1 on partition 32 (the PE
                    # col_grp granularity)
                    rt = ps_acc.tile([128, CH], F32, tag="r", name=f"ps_r_{qi}")
                    st["ps_rt"] = rt
                    st["ps_r"] = [rt[0:1, :], rt[32:33, :]]
                v_buf = st["v_buf"]
                e = [v_buf[:, kt * W + h * CH : kt * W + (h + 1) * CH]
                     for h in range(2)]
                st_, sp_ = (kt == 0), (kt == KT - 1)
                vck = vc_t[kt // GK][:, (kt % GK) * 128 : (kt % GK + 1) * 128]
                for h in halves:
                    nc.tensor.matmul(st["ps_r"][h], ones[:], e[h],
                                     start=st_, stop=sp_)
                for h in halves:
                    if h == 1 and halves == (0, 1):
                        # second PV half reuses the vc tile the first loaded
                        mm_reuse(st["ps_o"][1][:], vck, e[1], st_, sp_)
                    else:
                        nc.tensor.matmul(st["ps_o"][h][:], vck, e[h],
                                         start=st_, stop=sp_)

            def emit_out(qi):
                st = state.pop(qi)
                o_sb = outp.tile([128, W], F32, tag="o_sb", name=f"osb{qi}")
                r_sb = outp.tile([33, CH], F32, tag="r_sb", name=f"rsb{qi}")
                # one copy drains both row-sum rows (partitions 0 and 32);
                # per-half o DMAs overlap the second o copy
                nc.vector.tensor_copy(r_sb[:], st["ps_rt"][0:33, :])
                nc.vector.tensor_copy(o_sb[:, 0:CH], st["ps_o"][0][:])
                nc.sync.dma_start(ot_d.ap()[:, qi * W : qi * W + CH],
                                  o_sb[:, 0:CH])
                nc.sync.dma_start(r_d.ap()[:, qi * W : qi * W + CH],
                                  r_sb[0:1, :])
                nc.vector.tensor_copy(o_sb[:, CH : 2 * CH], st["ps_o"][1][:])
                nc.sync.dma_start(ot_d.ap()[:, qi * W + CH : (qi + 1) * W],
                                  o_sb[:, CH : 2 * CH])
                nc.scalar.dma_start(r_d.ap()[:, qi * W + CH : (qi + 1) * W],
                                    r_sb[32:33, :])

            # Flat software pipeline over all NITER*KT k-tile slots: the
            # fused-table pass lags the score matmuls by 2 slots, the
            # PV/row-sum matmuls by ~6 (8 for the first two tiles of each
            # iteration, giving the previous iteration's PSUM-accumulator
            # drain copies slack before the accumulators are reused), so
            # per-engine queues never stall at iteration boundaries.
            TOT = NITER * KT
            d_slot = {}
            for l in range(TOT - 1):
                kt = l % KT
                if l < KT:
                    # first iteration: no predecessor accumulator drain to
                    # dodge, but E production ramps with the ACT queue
                    lag = 8
                else:
                    lag = 10 if kt < 2 else 8
                d_slot.setdefault(l + lag, []).append(l)
            g_slot = {}
            for l in range(0, TOT - 2, 2):
                g_slot.setdefault(l + 2, []).append((l * W, 2 * W))
            # tail: last tile runs at half granularity so the final
            # sumsq -> g -> PV chain is as short as possible
            g_slot.setdefault(TOT, []).append(((TOT - 2) * W, W))
            g_slot.setdefault(TOT + 1, []).append(((TOT - 1) * W, CH))
            g_slot.setdefault(TOT + 1, []).append(((TOT - 1) * W + CH, CH))
            last = max(max(d_slot), max(g_slot))
            for j in range(last + 1):
                if j < TOT:
                    qi, kt = j // KT, j % KT
                    if kt == 0:
                        state[qi] = {
                            "v_buf": stage.tile([128, REG], F16, tag="v_buf",
                                                name=f"vbuf_{qi}")
                        }
                    emit_a_kt(qi, kt, state[qi]["v_buf"])
                for (c0, ncols) in g_slot.get(j, ()):
                    emit_g_cols(c0 // REG, c0 % REG, ncols)
                for l in d_slot.get(j, ()):
                    emit_d_kt(l // KT, l % KT)
                    if l % KT == KT - 1:
                        emit_out(l // KT)
            qL, ktL = (TOT - 1) // KT, (TOT - 1) % KT
            emit_d_kt(qL, ktL, halves=(0,))
            emit_d_kt(qL, ktL, halves=(1,))
            emit_out(qL)

    nc.compile()
    _CACHE["names"] = {
        "qc": f"qc_{tag}", "kc1": f"kc1_{tag}", "kc2": f"kc2_{tag}",
        "vc": f"vc_{tag}", "ot": f"ot_{tag}", "r": f"r_{tag}",
    }
    return nc


# ---------------------------------------------------------------- host packing
def pack_core(q_re, q_im, k_re, k_im, v_re, v_im):
    SK = k_re.shape[0]
    KT = SK // 128
    n = _CACHE["names"]
    qc = np.concatenate([q_re.T, q_im.T], axis=0).astype(np.float16)
    kc1 = np.concatenate([k_re.T, -k_im.T], axis=0).astype(np.float16)
    kc2 = np.concatenate([k_im.T, k_re.T], axis=0).astype(np.float16)
    vc = np.concatenate([v_re, v_im], axis=1).astype(np.float16).reshape(KT, 128, 128)
    return {n["qc"]: np.ascontiguousarray(qc), n["kc1"]: np.ascontiguousarray(kc1),
            n["kc2"]: np.ascontiguousarray(kc2), n["vc"]: np.ascontiguousarray(vc)}


def combine_host(parts):
    ot = sum(p[0].astype(np.float64) for p in parts)
    r = sum(p[1].reshape(-1).astype(np.float64) for p in parts)
    o = (ot / r[None, :]).astype(np.float32)
    return np.stack([o[0:D].T, o[D : 2 * D].T], axis=0)


# ---------------------------------------------------------------- harness entry
B, S = 4, 4096
SK_HALF = 2048


def _get_nc():
    if "nc" not in _CACHE:
        _CACHE["nc"] = build_kernel()
    return _CACHE["nc"]


def kernel(q_re, q_im, k_re, k_im, v_re, v_im, _trace=False):
    from concourse import bass_utils

    arrs = [np.asarray(a, dtype=np.float32)
            for a in (q_re, q_im, k_re, k_im, v_re, v_im)]
    assert arrs[0].shape == (B, S, D)

    nc = _get_nc()
    names = _CACHE["names"]
    maps = []
    for c in range(8):
        b, h = c // 2, c % 2
        ks = slice(h * SK_HALF, (h + 1) * SK_HALF)
        maps.append(pack_core(
            arrs[0][b], arrs[1][b],
            arrs[2][b, ks], arrs[3][b, ks],
            arrs[4][b, ks], arrs[5][b, ks]))
    res = None
    last_exc = None
    for attempt in range(3):
        try:
            res = bass_utils.run_bass_kernel_spmd(
                nc, maps, core_ids=list(range(8)), trace=_trace)
            break
        except Exception as e:  # transient device wedge: retry untraced
            last_exc = e
            _trace = False
    if res is None:
        raise last_exc
    out = np.empty((2, B, S, D), dtype=np.float32)
    for b in range(B):
        parts = [(res.results[2 * b + h][names["ot"]],
                  res.results[2 * b + h][names["r"]])
                 for h in range(2)]
        out[:, b] = combine_host(parts)
    if _trace:
        _CACHE["last_result"] = res
    return out


# revision 49
# speedup vs baseline: 1.1988x; 1.1988x over previous
"""Complex-valued dot-product attention (B=4, S=4096, D=64) on 8 TRN2 cores.

Harness entry: kernel(**inputs) -> np.ndarray [2, 4, 4096, 64] fp32.

Math (per batch): s = (q_re + i q_im)(k_re + i k_im)^T / 8,
w = softmax(|s|, axis=keys), out = stack(w @ v_re, w @ v_im).

Sharding: core c = (batch b = c//2, key half h = c%2). Each core computes a
partial softmax (flash-style) over its 2048 keys for all 4096 queries:
  OT = sum_k E^T [v_re|v_im],  R = sum_k E^T,  E = exp(|s|/8 - 6)
and the gather step combines o = (OT_0 + OT_1) / (R_0 + R_1) exactly (the
-6 bias is a shared constant so no per-shard max bookkeeping is needed).

Device kernel (per core): scores are built transposed, s^T [k_part, q_free],
via fp16 matmuls contracting 128 re/im-concatenated channels. Per k-tile a
custom fused DVE op (out = in0^2 + in1^2) drains both PSUM banks into
|s_raw|^2 in fp16. The whole  sqrt -> exp  chain then runs as a SINGLE
scalar-engine pass: the kernel compiles against a patched PWP activation
table in which the `exp` buckets are rewritten (same expansion points,
Taylor coefficients of g(t) = exp(sqrt(t) - 6)), so one ACT instruction
computes E = exp(|s_raw|/8 - 6) directly from |s_raw|^2 with scale=1/64.
The softmax row-sum (ones-matmul) and PV matmuls stream E^T back through
the tensor engine, PSUM-accumulated across the 16 k-tiles.
"""

import hashlib
import json
import os
import shutil

import numpy as np

import concourse.bacc as bacc
import concourse.bass as bass
import concourse.mybir as mybir
import concourse.tile as tile

F16 = mybir.dt.float16
F32 = mybir.dt.float32
AF = mybir.ActivationFunctionType

D = 64
SCALE = 1.0 / np.sqrt(np.float32(D))
EXP_BIAS = -6.0
M2_FLOOR = 2.0 ** -10  # keeps table input off the small-signal path

_OPS = {}
_CACHE = {}


# ------------------------------------------------------- custom DVE op
def _register(name, make_spec):
    from concourse import dve_ops
    from concourse.dve_ops import DveOp
    from concourse.dve_spec import lower, _has_src1
    from concourse.dve_uop import DveOpSpec

    if name in _OPS:
        return _OPS[name]
    for op in dve_ops.OPS:
        if op.name == name:
            _OPS[name] = op
            return op
    spec = make_spec()
    row = dve_ops._CUSTOM_DVE_ROW_BASE + len(dve_ops.OPS)
    assert row < 0x20
    dve_ops._SUB_OPCODE_FOR_NAME[name] = row
    shas = {}
    for ver in ("v3", "v4"):
        ds = DveOpSpec(
            name=name, opcode=row, uops=lower(spec, ver=ver), rd1_en=_has_src1(spec)
        )
        shas[ver] = ds.sha(ver)
    op = DveOp(name, spec, subdim=False, uops_sha=shas)
    dve_ops.OPS.append(op)
    dve_ops.CUSTOM_DVE_SPECS[name] = spec
    _OPS[name] = op
    return op


def register_sumsq():
    from concourse.dve_spec import Spec, Src0, Src1, sq

    return _register(
        "SUMSQ_ANT",
        lambda: Spec(
            body=sq(Src0) + Src1,
            reference=lambda in0, in1, s0, s1, imm2: (
                in0.astype(np.float32) ** 2 + in1.astype(np.float32)
            ),
        ),
    )


def register_sqdrain():
    from concourse.dve_spec import Spec, Src0, sq

    return _register(
        "SQDRAIN_ANT",
        lambda: Spec(
            body=sq(Src0),
            reference=lambda in0, in1, s0, s1, imm2: (
                in0.astype(np.float32) ** 2
            ),
        ),
    )


# ------------------------------------------------- patched activation table
# Rewrites every `exp` bucket (identified by its Taylor signature
# d0=e^a, d1=e^a, d2=e^a/2) so the scalar engine's Exp computes
# g(t) = exp(sqrt(t) - 6) for t > 0 and e^-6 for t <= 0.
def _patch_exp_buckets(raw):
    arr = np.frombuffer(raw, dtype=np.float32).reshape(-1, 8).copy()
    a = arr[:, 4].astype(np.float64)
    d0 = arr[:, 0].astype(np.float64)
    d1 = arr[:, 1].astype(np.float64)
    d2 = arr[:, 2].astype(np.float64)
    with np.errstate(over="ignore", invalid="ignore"):
        ea = np.exp(a)
        is_exp = (
            np.isfinite(ea)
            & (d0 > 0)
            & (np.abs(d0 - ea) <= 1e-5 * ea)
            & (np.abs(d1 - ea) <= 1e-5 * ea)
            & (np.abs(2.0 * d2 - ea) <= 1e-4 * ea)
        )
    pos = is_exp & (a > 0)
    A = a[pos]
    sA = np.sqrt(A)
    E = np.exp(sA + EXP_BIAS)
    arr[pos, 0] = E.astype(np.float32)
    arr[pos, 1] = (E / (2 * sA)).astype(np.float32)
    arr[pos, 2] = ((E * (1 / (4 * A) - 1 / (4 * A**1.5))) / 2.0).astype(np.float32)
    arr[pos, 3] = (
        (E * (1 / (8 * A**1.5) - 3 / (8 * A**2) + 3 / (8 * A**2.5))) / 6.0
    ).astype(np.float32)
    neg = is_exp & (a <= 0)
    arr[neg, 0] = np.float32(np.exp(EXP_BIAS))
    arr[neg, 1] = 0.0
    arr[neg, 2] = 0.0
    arr[neg, 3] = 0.0
    return arr.tobytes(), int(pos.sum())


def _build_table_dir():
    """Copy the compiler's default PWP table dir, patching exp buckets.

    Returns (table_dir, short content tag). Idempotent per content tag.
    """
    from neuronxcc.driver.Job import Job
    from neuronxcc.driver.jobs.support.FindActInfo import findActInfoFile

    src_json = os.environ.get("BASS_ACT_ROOT_JSON_PATH") or findActInfoFile(
        Job.getPackageDir(), "core_v4"
    )
    src = os.path.dirname(src_json)
    with open(src_json) as f:
        info = json.load(f)

    h = hashlib.sha256(b"expsqrt_v2")
    patches = {}
    n_sets = 0
    for s in info["act_func_sets"]:
        if "exp" not in s.get("act", {}):
            continue
        p = os.path.join(src, s["bkt_bin"])
        with open(p, "rb") as f:
            raw = f.read()
        patched, n_pos = _patch_exp_buckets(raw)
        assert n_pos > 100, f"{s['name']}: only {n_pos} exp buckets matched"
        patches[s["bkt_bin"]] = patched
        h.update(patched)
        n_sets += 1
    assert n_sets >= 1, "no exp-containing activation table sets found"
    tag = h.hexdigest()[:10]

    dst = f"/tmp/acttab_{tag}"
    if not os.path.exists(os.path.join(dst, os.path.basename(src_json))):
        tmp = dst + f".tmp{os.getpid()}"
        if os.path.exists(tmp):
            shutil.rmtree(tmp)
        shutil.copytree(src, tmp)
        os.chmod(tmp, 0o755)
        for fn in os.listdir(tmp):
            os.chmod(os.path.join(tmp, fn), 0o644)
        for fn, data in patches.items():
            with open(os.path.join(tmp, fn), "wb") as f:
                f.write(data)
        if os.path.exists(dst):
            shutil.rmtree(tmp)
        else:
            os.rename(tmp, dst)
    return os.path.join(dst, os.path.basename(src_json)), tag


# ---------------------------------------------------------------- device kernel
def build_kernel(SQ=4096, SK=2048, CH=512, dve_drain_mod=3):
    """dve_drain_mod: im^2 PSUM drains go to DVE (instead of ACT Square)
    for k-tiles with kt % dve_drain_mod == dve_drain_mod - 1. Balances the
    scalar and vector engines; dual-PSUM DVE reads are illegal so the
    re^2+im^2 combine always reads re from PSUM and im^2 from SBUF."""
    table_json, tag = _build_table_dir()
    os.environ["BASS_ACT_ROOT_JSON_PATH"] = table_json

    sumsq = register_sumsq()
    sqdrain = register_sqdrain()
    KT = SK // 128
    NITER = SQ // (2 * CH)
    W = 2 * CH
    REG = KT * W

    def mm_reuse(out, lhsT, rhs, start, stop):
        """Matmul that reuses the stationary operand loaded by the previous
        (self-loading) matmul — walrus skips the LDWEIGHTS re-emission."""
        eng = nc.tensor
        ifmap_ap = eng.lower_ap(rhs.opt(frozenset({0})), opt=False)
        weights_ap = eng.lower_ap(lhsT.opt(frozenset({0})), opt=False,
                                  for_matmul_weights=True)
        out_ap = eng.lower_ap(out)
        return eng.add_instruction(mybir.InstMatmult(
            name=nc.get_next_instruction_name(),
            replication_resolution=0, replication_shift_amnt=0,
            replication_num_rows=0,
            start_tensor_calc=start, stop_tensor_calc=stop,
            ins=[ifmap_ap, weights_ap], outs=[out_ap],
            ldweights=False,
            tile_position=(lhsT.base_partition(), out.base_partition()),
            tile_size=(128, round_up_pe(out.partition_size())),
        ))

    def round_up_pe(size):
        for v in (32, 64, 128):
            if v >= size:
                return v
        raise AssertionError(size)

    nc = bacc.Bacc("TRN2", target_bir_lowering=False)
    # Tensor names carry the table tag: the PJRT compile cache is keyed on
    # the BIR, and the activation table (env var) is not part of that key.
    qc_d = nc.dram_tensor(f"qc_{tag}", [128, SQ], F16, kind="ExternalInput")
    kc1_d = nc.dram_tensor(f"kc1_{tag}", [128, SK], F16, kind="ExternalInput")
    kc2_d = nc.dram_tensor(f"kc2_{tag}", [128, SK], F16, kind="ExternalInput")
    vc_d = nc.dram_tensor(f"vc_{tag}", [KT, 128, 128], F16, kind="ExternalInput")
    ot_d = nc.dram_tensor(f"ot_{tag}", [128, SQ], F32, kind="ExternalOutput")
    r_d = nc.dram_tensor(f"r_{tag}", [1, SQ], F32, kind="ExternalOutput")

    with tile.TileContext(nc) as tc:
        with (
            tc.tile_pool(name="singles", bufs=1) as singles,
            tc.tile_pool(name="stage", bufs=2) as stage,
            tc.tile_pool(name="outp", bufs=2) as outp,
            tc.tile_pool(name="ps_re", bufs=2, space="PSUM") as ps_re_pool,
            tc.tile_pool(name="ps_im", bufs=3, space="PSUM") as ps_im_pool,
            tc.tile_pool(name="ps_acc", bufs=1, space="PSUM") as ps_acc,
        ):
            # Inputs live in per-chunk tiles: tile-granular DMA dependencies
            # mean the first score matmul only waits for the first three
            # transfers, not a whole input tensor.
            GK = 4  # k-tiles per kc/vc chunk
            qc_t = [[singles.tile([128, CH], F16, name=f"qct{i}h{h}")
                     for h in range(2)] for i in range(NITER)]
            kc1_t = [singles.tile([128, GK * 128], F16, name=f"kc1t{i}")
                     for i in range(KT // GK)]
            kc2_t = [singles.tile([128, GK * 128], F16, name=f"kc2t{i}")
                     for i in range(KT // GK)]
            vc_t = [singles.tile([128, GK * 128], F16, name=f"vct{i}")
                    for i in range(KT // GK)]
            ones = singles.tile([128, 1], F16)
            ksl = lambda g: slice(g * GK * 128, (g + 1) * GK * 128)
            qsl = lambda i, h: slice(i * W + h * CH, i * W + (h + 1) * CH)
            # Input loads ordered by first use. Issuing a DMA costs the
            # issuing ENGINE ~600ns, so only the two earliest loads ride
            # the Scalar HW-DGE queue (Scalar is idle until the first
            # Square at ~9us); everything else stays on Sync.
            nc.sync.dma_start(kc2_t[0][:], kc2_d.ap()[:, ksl(0)])
            nc.scalar.dma_start(qc_t[0][0][:], qc_d.ap()[:, qsl(0, 0)])
            nc.scalar.dma_start(kc1_t[0][:], kc1_d.ap()[:, ksl(0)])
            nc.sync.dma_start(qc_t[0][1][:], qc_d.ap()[:, qsl(0, 1)])
            nc.scalar.dma_start(vc_t[0][:, 0:128], vc_d.ap()[0])
            for k in range(1, GK):
                nc.sync.dma_start(vc_t[0][:, k * 128 : (k + 1) * 128],
                                  vc_d.ap()[k])
            for g in range(1, KT // GK):
                nc.sync.dma_start(kc2_t[g][:], kc2_d.ap()[:, ksl(g)])
                nc.sync.dma_start(kc1_t[g][:], kc1_d.ap()[:, ksl(g)])
            for g in range(1, KT // GK):
                for k in range(GK):
                    nc.sync.dma_start(vc_t[g][:, k * 128 : (k + 1) * 128],
                                      vc_d.ap()[g * GK + k])
            for qi in range(1, NITER):
                for h in range(2):
                    nc.sync.dma_start(qc_t[qi][h][:], qc_d.ap()[:, qsl(qi, h)])
            nc.any.memset(ones[:], 1.0)
            g_bias = singles.tile([128, 1], F32)
            nc.any.memset(g_bias[:], M2_FLOOR)

            # PE warmup: the HAM clock gate defaults to 1.2 GHz and takes
            # ~3.4us of sustained activity to release. Burn tiny matmuls
            # into a scratch PSUM slot while the input DMAs stream so the
            # first real matmul runs at 2.4 GHz.
            warm_ps = ps_re_pool.tile([128, CH], F32, tag="re", name="warm_ps")
            for _ in range(40):
                nc.tensor.matmul(warm_ps[0:1, 0:1], ones[:, 0:1], ones[:, 0:1],
                                 start=True, stop=True)

            state = {}

            def emit_a_kt(qi, kt, v_buf):
                k_sl = slice((kt % GK) * 128, (kt % GK + 1) * 128)
                kc1k, kc2k = kc1_t[kt // GK], kc2_t[kt // GK]
                qch = [qc_t[qi][h][:] for h in range(2)]
                res, ims = [], []
                for half in range(2):
                    res.append(ps_re_pool.tile([128, CH], F32, tag="re",
                                               name=f"re_{qi}_{kt}_{half}"))
                    ims.append(ps_im_pool.tile([128, CH], F32, tag="im",
                                               name=f"im_{qi}_{kt}_{half}"))
                nc.tensor.matmul(ims[0][:], kc2k[:, k_sl], qch[0],
                                 start=True, stop=True)
                mm_reuse(ims[1][:], kc2k[:, k_sl], qch[1], True, True)
                nc.tensor.matmul(res[0][:], kc1k[:, k_sl], qch[0],
                                 start=True, stop=True)
                mm_reuse(res[1][:], kc1k[:, k_sl], qch[1], True, True)
                dve_drain = (kt % dve_drain_mod) == dve_drain_mod - 1
                for half in range(2):
                    reg = slice(kt * W + half * CH, kt * W + (half + 1) * CH)
                    if dve_drain:
                        nc.vector._custom_dve(
                            sqdrain, out=v_buf[:, reg], in0=ims[half][:]
                        )
                    else:
                        nc.scalar.activation(v_buf[:, reg], ims[half][:], AF.Square)
                    nc.vector._custom_dve(
                        sumsq, out=v_buf[:, reg], in0=res[half][:], in1=v_buf[:, reg]
                    )

            def emit_g_cols(qi, c0, ncols):
                # E = exp(sqrt(m2/64 + floor) - 6) via the patched exp table
                v_buf = state[qi]["v_buf"]
                sl = slice(c0, c0 + ncols)
                nc.scalar.activation(
                    v_buf[:, sl], v_buf[:, sl], AF.Exp,
                    scale=float(SCALE * SCALE), bias=g_bias[:],
                )

            def emit_d_kt(qi, kt, halves=(0, 1)):
                st = state[qi]
                if "ps_o" not in st:
                    st["ps_o"] = [
                        ps_acc.tile([128, CH], F32, tag="oA", name=f"ps_oA_{qi}"),
                        ps_acc.tile([128, CH], F32, tag="oB", name=f"ps_oB_{qi}"),
                    ]
                    # both row-sum accumulators share one PSUM bank: half 0
                    # lands on partition 0, half 1 on partition 32 (the PE
                    # col_grp granularity)
                    rt = ps_acc.tile([128, CH], F32, tag="r", name=f"ps_r_{qi}")
                    st["ps_rt"] = rt
                    st["ps_r"] = [rt[0:1, :], rt[32:33, :]]
                v_buf = st["v_buf"]
                e = [v_buf[:, kt * W + h * CH : kt * W + (h + 1) * CH]
                     for h in range(2)]
                st_, sp_ = (kt == 0), (kt == KT - 1)
                vck = vc_t[kt // GK][:, (kt % GK) * 128 : (kt % GK + 1) * 128]
                for h in halves:
                    nc.tensor.matmul(st["ps_r"][h], ones[:], e[h],
                                     start=st_, stop=sp_)
                for h in halves:
                    if h == 1 and halves == (0, 1):
                        # second PV half reuses the vc tile the first loaded
                        mm_reuse(st["ps_o"][1][:], vck, e[1], st_, sp_)
                    else:
                        nc.tensor.matmul(st["ps_o"][h][:], vck, e[h],
                                         start=st_, stop=sp_)

            def emit_out(qi):
                st = state.pop(qi)
                o_sb = outp.tile([128, W], F32, tag="o_sb", name=f"osb{qi}")
                r_sb = outp.tile([33, CH], F32, tag="r_sb", name=f"rsb{qi}")
                # one copy drains both row-sum rows (partitions 0 and 32);
                # per-half o DMAs overlap the second o copy
                nc.vector.tensor_copy(r_sb[:], st["ps_rt"][0:33, :])
                nc.vector.tensor_copy(o_sb[:, 0:CH], st["ps_o"][0][:])
                nc.sync.dma_start(ot_d.ap()[:, qi * W : qi * W + CH],
                                  o_sb[:, 0:CH])
                nc.sync.dma_start(r_d.ap()[:, qi * W : qi * W + CH],
                                  r_sb[0:1, :])
                nc.vector.tensor_copy(o_sb[:, CH : 2 * CH], st["ps_o"][1][:])
                nc.sync.dma_start(ot_d.ap()[:, qi * W + CH : (qi + 1) * W],
                                  o_sb[:, CH : 2 * CH])
                nc.scalar.dma_start(r_d.ap()[:, qi * W + CH : (qi + 1) * W],
                                    r_sb[32:33, :])

            # Flat software pipeline over all NITER*KT k-tile slots: the
            # fused-table pass lags the score matmuls by 2 slots, the
            # PV/row-sum matmuls by ~6 (8 for the first two tiles of each
            # iteration, giving the previous iteration's PSUM-accumulator
            # drain copies slack before the accumulators are reused), so
            # per-engine queues never stall at iteration boundaries.
            TOT = NITER * KT
            d_slot = {}
            for l in range(TOT - 1):
                kt = l % KT
                if l < KT:
                    # first iteration: no predecessor accumulator drain to
                    # dodge, but E production ramps with the ACT queue
                    lag = 8
                else:
                    lag = 10 if kt < 2 else 8
                d_slot.setdefault(l + lag, []).append(l)
            g_slot = {}
            for l in range(0, TOT - 2, 2):
                g_slot.setdefault(l + 2, []).append((l * W, 2 * W))
            # tail: last tile runs at half granularity so the final
            # sumsq -> g -> PV chain is as short as possible
            g_slot.setdefault(TOT, []).append(((TOT - 2) * W, W))
            g_slot.setdefault(TOT + 1, []).append(((TOT - 1) * W, CH))
            g_slot.setdefault(TOT + 1, []).append(((TOT - 1) * W + CH, CH))
            last = max(max(d_slot), max(g_slot))
            for j in range(last + 1):
                if j < TOT:
                    qi, kt = j // KT, j % KT
                    if kt == 0:
                        state[qi] = {
                            "v_buf": stage.tile([128, REG], F16, tag="v_buf",
                                                name=f"vbuf_{qi}")
                        }
                    emit_a_kt(qi, kt, state[qi]["v_buf"])
                for (c0, ncols) in g_slot.get(j, ()):
                    emit_g_cols(c0 // REG, c0 % REG, ncols)
                for l in d_slot.get(j, ()):
                    emit_d_kt(l // KT, l % KT)
                    if l % KT == KT - 1:
                        emit_out(l // KT)
            qL, ktL = (TOT - 1) // KT, (TOT - 1) % KT
            emit_d_kt(qL, ktL, halves=(0,))
            emit_d_kt(qL, ktL, halves=(1,))
            emit_out(qL)

    nc.compile()
    _CACHE["names"] = {
        "qc": f"qc_{tag}", "kc1": f"kc1_{tag}", "kc2": f"kc2_{tag}",
        "vc": f"vc_{tag}", "ot": f"ot_{tag}", "r": f"r_{tag}",
    }
    return nc


# ---------------------------------------------------------------- host packing
def pack_core(q_re, q_im, k_re, k_im, v_re, v_im):
    SK = k_re.shape[0]
    KT = SK // 128
    n = _CACHE["names"]
    qc = np.concatenate([q_re.T, q_im.T], axis=0).astype(np.float16)
    kc1 = np.concatenate([k_re.T, -k_im.T], axis=0).astype(np.float16)
    kc2 = np.concatenate([k_im.T, k_re.T], axis=0).astype(np.float16)
    vc = np.concatenate([v_re, v_im], axis=1).astype(np.float16).reshape(KT, 128, 128)
    return {n["qc"]: np.ascontiguousarray(qc), n["kc1"]: np.ascontiguousarray(kc1),
            n["kc2"]: np.ascontiguousarray(kc2), n["vc"]: np.ascontiguousarray(vc)}


def combine_host(parts):
    ot = sum(p[0].astype(np.float64) for p in parts)
    r = sum(p[1].reshape(-1).astype(np.float64) for p in parts)
    o = (ot / r[None, :]).astype(np.float32)
    return np.stack([o[0:D].T, o[D : 2 * D].T], axis=0)


# ---------------------------------------------------------------- harness entry
B, S = 4, 4096
SK_HALF = 2048


def _get_nc():
    if "nc" not in _CACHE:
        _CACHE["nc"] = build_kernel()
    return _CACHE["nc"]


def kernel(q_re, q_im, k_re, k_im, v_re, v_im, _trace=False):
    from concourse import bass_utils

    arrs = [np.asarray(a, dtype=np.float32)
            for a in (q_re, q_im, k_re, k_im, v_re, v_im)]
    assert arrs[0].shape == (B, S, D)

    nc = _get_nc()
    names = _CACHE["names"]
    maps = []
    for c in range(8):
        b, h = c // 2, c % 2
        ks = slice(h * SK_HALF, (h + 1) * SK_HALF)
        maps.append(pack_core(
            arrs[0][b], arrs[1][b],
            arrs[2][b, ks], arrs[3][b, ks],
            arrs[4][b, ks], arrs[5][b, ks]))
    res = None
    last_exc = None
    for attempt in range(3):
        try:
            res = bass_utils.run_bass_kernel_spmd(
                nc, maps, core_ids=list(range(8)), trace=_trace)
            break
        except Exception as e:  # transient device wedge: retry untraced
            last_exc = e
            _trace = False
    if res is None:
        raise last_exc
    out = np.empty((2, B, S, D), dtype=np.float32)
    for b in range(B):
        parts = [(res.results[2 * b + h][names["ot"]],
                  res.results[2 * b + h][names["r"]])
                 for h in range(2)]
        out[:, b] = combine_host(parts)
    if _trace:
        _CACHE["last_result"] = res
    return out
